# revision 18
# baseline (speedup 1.0000x reference)
"""Trainium2 Bass kernel for nn_CViTFlow (cross-attention ViT flow block).

Math (per the module):
  two token streams x1,x2 [B,T,256] viewed as [B,256,48,48] images.
  6 branches (q1,k1,v1,q2,k2,v2): depthwise3x3 -> BN(eval) -> 1x1 conv -> Linear.
  o1 = softmax(-(q1 k2^T / 16)) v2 + q1 ;  o2 = softmax(-(q2 k1^T / 16)) v1 + q2
  both reshaped [B,H,T,DH] -> [B,T,256] with a plain (head-major) reshape.

Kernel strategy (v2 — full-array attention, HAM-warm):
  * Host folds BN + 1x1conv + Linear into one 256x256 matrix W and bias c per
    branch, then folds the depthwise 3x3 into 9 "tap" matrices, so a branch is
    9 shifted matmuls accumulated in PSUM (dense K=128/M=128 on the PE).
  * 8 cores = (map m) x (batch b) x (head-quad g); no collectives.
  * Attention is restructured so every matmul uses the FULL 128x128 array
    (the PE_HAM clock gate throttles the PE to 1.2 GHz when it doesn't see
    dense activity — the previous kernel ran its whole attention phase at
    half clock):
      - scores: stationary = kT 128-chunk [128=(4h x 32d), 128 t] (dense),
        stream = qblk, a block-diagonal copy of qT (head h's rows only in
        block h, zeros elsewhere). Cross-head terms vanish exactly, giving
        sc[t, (h,l)] per-head scores with full-K matmuls.
      - AV is flipped: stationary = exp tile [128 t, 128 l] (dense), stream =
        v chunk [128 t, 32+1 cols] (ones col -> softmax denominator), so the
        big operand rides the weight path (LDW) and the stream is tiny.
        Output lands ^T as [l, (h,c)]; the host un-transposes.
  * exp on ScalarE ([128,1024] per chunk) is the pipeline's critical engine
    (~(1024+352)/1.2 ns each); the branch phase is interleaved with l-tile 0
    of attention so ScalarE starts ~20us earlier.
  * q residual is DMA'd out in f32 and added on the host.
"""

import numpy as np

B = 2
T = 2304
DIM = 256
HEADS = 8
DH = 32
HW = 48
EPS = 1e-5
P = 128
N_CORES = 8

# t-tiles for the branch phase: row-aligned in the 48x48 image (10/8 rows)
T_TILES = [(0, 480, 0, 10), (480, 480, 10, 10), (960, 480, 20, 10),
           (1440, 480, 30, 10), (1920, 384, 40, 8)]
NL = 256            # l-tile width (2 x 128 sub-tiles)
N_LT = T // NL      # 9 l-tiles
N_TCH = T // P      # 18 t-chunks of 128
# exp tiles in flight: big enough that ScalarE (the bottleneck engine) can
# run well ahead of the AV consumers while the q/v branches finish on the PE
ET_BUFS = 26

_PROGRAM = None
_last_in_maps = None


def _build_program(debug=False):
    """Build the SPMD Bass/Tile program (identical for all 8 cores)."""
    from contextlib import ExitStack

    import concourse.bacc as bacc
    import concourse.mybir as mybir
    import concourse.tile as tile
    from concourse.masks import make_identity
    from concourse.tile_rust import add_dep_helper

    f32 = mybir.dt.float32
    bf16 = mybir.dt.bfloat16
    AF = mybir.ActivationFunctionType

    nc = bacc.Bacc(None, target_bir_lowering=False, debug=False)

    pad_a = nc.declare_dram_parameter("pad_a", [2, P, 2500], bf16, isOutput=False)
    pad_b = nc.declare_dram_parameter("pad_b", [2, P, 2500], bf16, isOutput=False)
    wq = nc.declare_dram_parameter("wq", [2, P, 9 * P], bf16, isOutput=False)
    wk = nc.declare_dram_parameter("wk", [2, P, 9 * P], bf16, isOutput=False)
    wv = nc.declare_dram_parameter("wv", [2, P, 9 * P], bf16, isOutput=False)
    bias_d = nc.declare_dram_parameter("bias", [3, P, 1], f32, isOutput=False)
    # attention part, ^T layout: col = li*256 + lsub*128 + h*32 + c, row = l%128
    out_d = nc.declare_dram_parameter("out", [P, T], f32, isOutput=True)
    qout_d = nc.declare_dram_parameter("qout", [P, T], f32, isOutput=True)

    with tile.TileContext(nc) as tc, ExitStack() as ctx:
        const = ctx.enter_context(tc.tile_pool(name="const", bufs=1))
        sb = ctx.enter_context(tc.tile_pool(name="sb", bufs=1))
        fin = ctx.enter_context(tc.tile_pool(name="fin", bufs=2))

        identity = const.tile([P, P], bf16)
        make_identity(nc, identity)

        # ---- input DMAs (k weights + image B first: they gate phase A) ----
        wk_sb = sb.tile([P, 2 * 9 * P], bf16, tag="wk")
        pb_sb = sb.tile([P, 2 * 2500], bf16, tag="pb")
        wq_sb = sb.tile([P, 2 * 9 * P], bf16, tag="wq")
        pa_sb = sb.tile([P, 2 * 2500], bf16, tag="pa")
        wv_sb = sb.tile([P, 2 * 9 * P], bf16, tag="wv")
        bias_sb = sb.tile([P, 3], f32, tag="bias")
        # first k-tile needs: all of wk + image-B rows 0-11 (cols < 600).
        # Front-load exactly that so the PE starts ~5us earlier.
        for kc in range(2):
            nc.sync.dma_start(wk_sb[:, kc * 1152:(kc + 1) * 1152], wk[kc])
            nc.sync.dma_start(pb_sb[:, kc * 2500:kc * 2500 + 600],
                              pad_b[kc][:, 0:600])
        for kc in range(2):
            nc.sync.dma_start(wq_sb[:, kc * 1152:(kc + 1) * 1152], wq[kc])
            nc.sync.dma_start(pa_sb[:, kc * 2500:kc * 2500 + 600],
                              pad_a[kc][:, 0:600])
        for r in range(3):
            nc.sync.dma_start(bias_sb[:, r:r + 1], bias_d[r])
        for kc in range(2):
            nc.sync.dma_start(pb_sb[:, kc * 2500 + 600:(kc + 1) * 2500],
                              pad_b[kc][:, 600:2500])
            nc.sync.dma_start(pa_sb[:, kc * 2500 + 600:(kc + 1) * 2500],
                              pad_a[kc][:, 600:2500])
        for kc in range(2):
            nc.sync.dma_start(wv_sb[:, kc * 1152:(kc + 1) * 1152], wv[kc])

        kT = sb.tile([P, T], bf16, tag="kT")
        qT = sb.tile([P, T], bf16, tag="qT")
        qTf = sb.tile([P, T], f32, tag="qTf")
        vT = sb.tile([P, T], bf16, tag="vT")
        # block-diagonal q for dense scores: head h lives at rows 32h:32h+32,
        # cols h*T + l; all other rows zero.
        qblk = sb.tile([P, 4 * T], bf16, tag="qblk")
        # v stream for flipped AV: per (chunk j, head h) 33 cols = [v(32)|ones]
        vstr = sb.tile([P, N_TCH * 4 * 33], bf16, tag="vstr")
        outT = sb.tile([P, T], f32, tag="outT")
        ep = ctx.enter_context(tc.tile_pool(name="ep", bufs=2))

        # zero qblk during the input-DMA wait (its off-head rows must be 0)
        nc.vector.memset(qblk[:], 0.0)
        ones_cols = vstr.rearrange("p (b c) -> p b c", c=33)[:, :, 32:33]
        nc.vector.memset(ones_cols, 1.0)

        psum = ctx.enter_context(tc.tile_pool(name="psum", bufs=2, space="PSUM"))

        # ================= Phase A: branch matmuls =================
        def branch_tile(w_sb, img_sb, dest, role, tt, dest2=None):
            (t0, nt, r0, nr) = tt
            ps = psum.tile([P, 1024], f32, tag="sc", bufs=3,
                           name=f"br_{role}_{t0}")
            mm = 0
            for kc in range(2):
                pv = img_sb[:, kc * 2500:(kc + 1) * 2500].rearrange(
                    "p (r c) -> p r c", c=50)
                wv_ = w_sb[:, kc * 1152:(kc + 1) * 1152]
                for di in range(3):
                    for dj in range(3):
                        tap = di * 3 + dj
                        rhs = pv[:, r0 + di:r0 + di + nr, dj:dj + 48]
                        nc.tensor.matmul(
                            ps[:, 0:nt], wv_[:, tap * P:(tap + 1) * P], rhs,
                            start=(mm == 0), stop=(mm == 17))
                        mm += 1
            nc.vector.tensor_scalar_add(dest[:, t0:t0 + nt], ps[:, 0:nt],
                                        bias_sb[:, role:role + 1])
            if dest2 is not None:
                nc.vector.tensor_scalar_add(dest2[:, t0:t0 + nt], ps[:, 0:nt],
                                            bias_sb[:, role:role + 1])

        def qblk_dma(tt):
            (t0, nt, _, _) = tt
            for h in range(4):
                nc.sync.dma_start(
                    qblk[h * 32:(h + 1) * 32, h * T + t0:h * T + t0 + nt],
                    qT[h * 32:(h + 1) * 32, t0:t0 + nt])

        def v_transpose(j):
            tp = psum.tile([P, P], bf16, tag="sc", bufs=3, name=f"tp_{j}")
            nc.tensor.transpose(tp[:], vT[:, j * P:(j + 1) * P], identity[:])
            dst = vstr[:, j * 132:(j + 1) * 132].rearrange(
                "p (h c) -> p h c", c=33)[:, :, 0:32]
            src = tp.rearrange("p (h c) -> p h c", c=32)
            nc.vector.tensor_copy(dst, src)

        # ================= Phase B: attention =================
        sc_tiles = {}
        et_tiles = {}
        qv = qblk.rearrange("p (h l) -> p h l", l=T)

        def scores(li, j):
            l0 = li * NL
            sc = psum.tile([P, 4 * NL], f32, tag="sc", bufs=3,
                           name=f"sc_{li}_{j}")
            kch = kT[:, j * P:(j + 1) * P]
            nc.tensor.matmul(sc[:, 0:512], kch, qv[:, 0:2, l0:l0 + NL],
                             start=True, stop=True)
            last = nc.tensor.matmul(sc[:, 512:1024], kch,
                                    qv[:, 2:4, l0:l0 + NL],
                                    start=True, stop=True)
            sc_tiles[(li, j)] = sc
            return last

        def exp_chunk(li, j):
            sc = sc_tiles.pop((li, j))
            et = ep.tile([P, 4 * NL], bf16, tag="e", bufs=ET_BUFS,
                         name=f"e_{li}_{j}")
            nc.scalar.activation(et[:], sc[:], AF.Exp, scale=-0.0625)
            et_tiles[(li, j)] = et

        def av_chunk(li, j, avp, dep=None):
            et = et_tiles.pop((li, j))
            for h in range(4):
                for ls in range(2):
                    av = nc.tensor.matmul(
                        avp[:, (ls * 4 + h) * 33:(ls * 4 + h) * 33 + 33],
                        et[:, h * NL + ls * P:h * NL + ls * P + P],
                        vstr[:, (j * 4 + h) * 33:(j * 4 + h) * 33 + 33],
                        start=(j == 0), stop=(j == N_TCH - 1),
                        skip_group_check=True)
                    if dep is not None:
                        # keep next scores AHEAD of this exp-gated AV in the
                        # in-order PE stream (scheduling-only ordering edge)
                        add_dep_helper(av.ins, dep.ins,
                                       reason="scores(t+1) before AV(t)")
                        dep = None

        def finalize(li, avp):
            # one strided reciprocal for all 8 denominators (ones cols)
            rc8 = fin.tile([P, 8], f32, tag="recip", bufs=2, name=f"rc_{li}")
            den = avp.rearrange("p (b c) -> p b c", c=33)[:, :, 32:33]
            nc.vector.reciprocal(rc8.rearrange("p (b o) -> p b o", o=1), den)
            for ls in range(2):
                for h in range(4):
                    base = (ls * 4 + h) * 33
                    nc.vector.tensor_scalar_mul(
                        outT[:, li * NL + ls * P + h * 32:
                             li * NL + ls * P + h * 32 + 32],
                        avp[:, base:base + 32],
                        rc8[:, ls * 4 + h:ls * 4 + h + 1])
            nc.sync.dma_start(out_d[:, li * NL:(li + 1) * NL],
                              outT[:, li * NL:(li + 1) * NL])

        # ---------------- schedule ----------------
        # Unified score/AV cursors over flat chunk index c = li*18 + j.
        # Scores for chunk c become available once the k tiles covering t
        # chunk j and the q tiles covering l-tile li are done; they are
        # emitted greedily (up to ET_BUFS ahead of AV) so ScalarE — the
        # critical engine at ~1.1us per exp chunk — starts ~22us in and
        # never starves while the PE finishes the q/v branches.
        NCH = N_LT * N_TCH
        state = {"sc": 0, "av": 0, "khi": -1, "qhi": -1, "dep": None}
        av_tiles = {}

        def emit_scores_avail(max_n=NCH):
            # paced emission: a branch unit is ~3.6us of PE work and ScalarE
            # drains ~1.1us/chunk, so callers cap at ~4 per unit — emitting
            # greedily would head-of-line-block the PE on sc-ring waits
            n = 0
            while (state["sc"] < NCH and n < max_n
                   and (state["sc"] - state["av"]) < ET_BUFS):
                li, j = divmod(state["sc"], N_TCH)
                if li > state["qhi"] or j > state["khi"]:
                    return
                state["dep"] = scores(li, j)
                exp_chunk(li, j)
                state["sc"] += 1
                n += 1

        def emit_av():
            li, j = divmod(state["av"], N_TCH)
            if j == 0:
                av_tiles[li] = psum.tile([P, 264], f32, tag="av", bufs=2,
                                         name=f"avp_{li}")
            av_chunk(li, j, av_tiles[li], dep=state["dep"])
            state["dep"] = None
            state["av"] += 1
            if j == 4 and li > 0:
                finalize(li - 1, av_tiles.pop(li - 1))

        state["tp"] = -1  # highest v transpose emitted

        # Phase A head: k tile 0, then a 288-col q head tile (covers l-tile
        # 0's 256 stream cols) whose qblk DMA latency hides under k tile 1;
        # the first scores issue ~24us in — ScalarE, the critical engine,
        # starts there. Scores for later t-chunks unlock per k tile (khi).
        K_HI = [2, 6, 10, 14, 17]
        branch_tile(wk_sb, pb_sb, kT, 1, T_TILES[0])
        state["khi"] = K_HI[0]
        branch_tile(wq_sb, pa_sb, qT, 0, (0, 288, 0, 6), dest2=qTf)
        qblk_dma((0, 288, 0, 6))
        state["qhi"] = 0
        for ti in range(1, 5):
            branch_tile(wk_sb, pb_sb, kT, 1, T_TILES[ti])
            state["khi"] = K_HI[ti]
            emit_scores_avail(3)

        # Remaining q/v branch work, split into HALF tiles (~1.8us of PE
        # work each) so the sc-ring backlog (bufs=3, ~3.3us of ScalarE
        # runway) covers each injected burst without the exp stream running
        # dry. v halves are injected when the next AV chunk is blocked on a
        # missing transpose; q halves predictively, 8 chunks before the
        # scores cursor would block on l-tile coverage.
        # ~1.1us (3-row) tiles: small enough that the ScalarE queue
        # (ring-limited to ~2-3 chunks) absorbs each injected PE burst
        def row_tiles(r_start):
            out = []
            r = r_start
            while r < HW:
                nr = min(3, HW - r)
                out.append((r * HW, nr * HW, r, nr))
                r += nr
            return out
        v_units = [(tt, (tt[0] + tt[1]) // P - 1) for tt in row_tiles(0)]
        q_units = [(tt, (tt[0] + tt[1]) // NL - 1)
                   for tt in [(288, 192, 6, 4)] + row_tiles(10)]

        def inject_v():
            tt, tpmax = v_units.pop(0)
            branch_tile(wv_sb, pb_sb, vT, 2, tt)
            while state["tp"] < tpmax:
                state["tp"] += 1
                v_transpose(state["tp"])

        def inject_q():
            tt, qhi = q_units.pop(0)
            branch_tile(wq_sb, pa_sb, qT, 0, tt, dest2=qTf)
            qblk_dma(tt)
            state["qhi"] = qhi
            if not q_units:
                nc.sync.dma_start(qout_d[:], qTf[:])

        while state["av"] < NCH:
            # scores first, whenever available — ScalarE is the bottleneck
            # and the sc-ring (bufs=3) paces the PE against it; the et ring
            # (26) absorbs the run-ahead while AVs wait on v transposes
            emit_scores_avail(2)
            if q_units and state["sc"] >= (state["qhi"] + 1) * N_TCH - 8:
                inject_q()
            li, j = divmod(state["av"], N_TCH)
            if li == 0 and j > state["tp"] and v_units:
                inject_v()
            elif state["av"] < state["sc"]:
                emit_av()
            elif v_units:
                inject_v()
            elif q_units:
                inject_q()
        finalize(N_LT - 1, av_tiles.pop(N_LT - 1))

        if debug:
            for nm, t in [("dbg_qT", qT), ("dbg_kT", kT), ("dbg_vT", vT),
                          ("dbg_vstr", vstr), ("dbg_qblk", qblk),
                          ("dbg_qTf", qTf)]:
                dd = nc.declare_dram_parameter(nm, list(t.shape), t.dtype,
                                               isOutput=True)
                nc.sync.dma_start(dd[:], t[:])

    nc.compile()
    return nc


def _fold_weights(dw_w, bn_gamma, bn_beta, bn_mean, bn_var, pw_w, pw_b, lin_w):
    """Fold BN + pointwise conv + linear (+ depthwise taps) per branch.

    Returns Wtap [6, 9, 256, 256] (float32) and bias c [6, 256]."""
    dw = dw_w.astype(np.float64)
    g = bn_gamma.astype(np.float64)
    b = bn_beta.astype(np.float64)
    m = bn_mean.astype(np.float64)
    v = bn_var.astype(np.float64)
    pw = pw_w.astype(np.float64)
    pb = pw_b.astype(np.float64)
    lw = lin_w.astype(np.float64)

    scale = g / np.sqrt(v + EPS)
    shift = b - m * scale
    M = np.einsum("noc,ncd->nod", lw, pw)
    W = M * scale[:, None, :]
    c = np.einsum("noc,nc->no", M, shift) + np.einsum("noc,nc->no", lw, pb)
    Wtap = W[:, None, :, :] * dw.transpose(0, 2, 3, 1).reshape(6, 9, 1, 256)
    return Wtap.astype(np.float32), c.astype(np.float32)


def _bf16(a):
    import ml_dtypes
    return a.astype(ml_dtypes.bfloat16)


def _pad_images(x):
    """x [B, T, 256] -> per batch channel-major zero-padded bf16 [2,128,2500]."""
    out = np.zeros((B, 2, P, 50, 50), dtype=np.float32)
    img = np.ascontiguousarray(x.transpose(0, 2, 1)).reshape(B, DIM, HW, HW)
    out[:, :, :, 1:49, 1:49] = img.reshape(B, 2, P, HW, HW)
    return _bf16(out.reshape(B, 2, P, 2500))


def _wtap_lhsT(Wtap, branch, g):
    """Pack lhsT layout [2, 128, 9*128] for a branch restricted to quad g."""
    rows = slice(g * P, (g + 1) * P)
    out = np.empty((2, P, 9 * P), dtype=np.float32)
    for kc in range(2):
        for tap in range(9):
            blk = Wtap[branch, tap][rows, kc * P:(kc + 1) * P]
            out[kc, :, tap * P:(tap + 1) * P] = blk.T
    return _bf16(out)


def kernel(x1, x2, dw_w, bn_gamma, bn_beta, bn_mean, bn_var, pw_w, pw_b, lin_w,
           h1=HW, w1=HW, h2=HW, w2=HW):
    global _PROGRAM
    from concourse.bass_utils import run_bass_kernel_spmd

    x1 = np.asarray(x1, dtype=np.float32)
    x2 = np.asarray(x2, dtype=np.float32)

    Wtap, c = _fold_weights(np.asarray(dw_w), np.asarray(bn_gamma),
                            np.asarray(bn_beta), np.asarray(bn_mean),
                            np.asarray(bn_var), np.asarray(pw_w),
                            np.asarray(pw_b), np.asarray(lin_w))
    pad1 = _pad_images(x1)
    pad2 = _pad_images(x2)

    if _PROGRAM is None:
        _PROGRAM = _build_program()
    nc = _PROGRAM

    # core layout: core = m*4 + b*2 + g
    in_maps = []
    for m in range(2):
        qbr, kbr, vbr = (0, 4, 5) if m == 0 else (3, 1, 2)
        pa, pb_ = (pad1, pad2) if m == 0 else (pad2, pad1)
        for b in range(2):
            for g in range(2):
                bias = np.stack([c[qbr, g * P:(g + 1) * P],
                                 c[kbr, g * P:(g + 1) * P],
                                 c[vbr, g * P:(g + 1) * P]])[:, :, None]
                in_maps.append({
                    "pad_a": np.ascontiguousarray(pa[b]),
                    "pad_b": np.ascontiguousarray(pb_[b]),
                    "wq": _wtap_lhsT(Wtap, qbr, g),
                    "wk": _wtap_lhsT(Wtap, kbr, g),
                    "wv": _wtap_lhsT(Wtap, vbr, g),
                    "bias": np.ascontiguousarray(bias),
                })

    global _last_in_maps
    _last_in_maps = in_maps
    res = run_bass_kernel_spmd(nc, in_maps, list(range(N_CORES)))

    o = np.empty((2, 2, HEADS, T, DH), dtype=np.float32)
    for m in range(2):
        for b in range(2):
            for g in range(2):
                core = m * 4 + b * 2 + g
                att = res.results[core]["out"]
                qf = res.results[core]["qout"]
                a = att.reshape(P, N_LT, 2, 4, DH).transpose(1, 2, 0, 3, 4)
                a = a.reshape(T, 4, DH)
                qq = qf.reshape(4, DH, T).transpose(2, 0, 1)
                o[m, b, 4 * g:4 * g + 4] = (a + qq).transpose(1, 0, 2)
    o1 = o[0].reshape(B, T, HEADS * DH)
    o2 = o[1].reshape(B, T, HEADS * DH)
    return o1, o2


# revision 19
# speedup vs baseline: 1.0038x; 1.0038x over previous
"""Trainium2 Bass kernel for nn_CViTFlow (cross-attention ViT flow block).

Math (per the module):
  two token streams x1,x2 [B,T,256] viewed as [B,256,48,48] images.
  6 branches (q1,k1,v1,q2,k2,v2): depthwise3x3 -> BN(eval) -> 1x1 conv -> Linear.
  o1 = softmax(-(q1 k2^T / 16)) v2 + q1 ;  o2 = softmax(-(q2 k1^T / 16)) v1 + q2
  both reshaped [B,H,T,DH] -> [B,T,256] with a plain (head-major) reshape.

Kernel strategy (v2 — full-array attention, HAM-warm):
  * Host folds BN + 1x1conv + Linear into one 256x256 matrix W and bias c per
    branch, then folds the depthwise 3x3 into 9 "tap" matrices, so a branch is
    9 shifted matmuls accumulated in PSUM (dense K=128/M=128 on the PE).
  * 8 cores = (map m) x (batch b) x (head-quad g); no collectives.
  * Attention is restructured so every matmul uses the FULL 128x128 array
    (the PE_HAM clock gate throttles the PE to 1.2 GHz when it doesn't see
    dense activity — the previous kernel ran its whole attention phase at
    half clock):
      - scores: stationary = kT 128-chunk [128=(4h x 32d), 128 t] (dense),
        stream = qblk, a block-diagonal copy of qT (head h's rows only in
        block h, zeros elsewhere). Cross-head terms vanish exactly, giving
        sc[t, (h,l)] per-head scores with full-K matmuls.
      - AV is flipped: stationary = exp tile [128 t, 128 l] (dense), stream =
        v chunk [128 t, 32+1 cols] (ones col -> softmax denominator), so the
        big operand rides the weight path (LDW) and the stream is tiny.
        Output lands ^T as [l, (h,c)]; the host un-transposes.
  * exp on ScalarE ([128,1024] per chunk) is the pipeline's critical engine
    (~(1024+352)/1.2 ns each); the branch phase is interleaved with l-tile 0
    of attention so ScalarE starts ~20us earlier.
  * q residual is DMA'd out in f32 and added on the host.
"""

import numpy as np

B = 2
T = 2304
DIM = 256
HEADS = 8
DH = 32
HW = 48
EPS = 1e-5
P = 128
N_CORES = 8

# t-tiles for the branch phase: row-aligned in the 48x48 image (10/8 rows)
T_TILES = [(0, 480, 0, 10), (480, 480, 10, 10), (960, 480, 20, 10),
           (1440, 480, 30, 10), (1920, 384, 40, 8)]
NL = 256            # l-tile width (2 x 128 sub-tiles)
N_LT = T // NL      # 9 l-tiles
N_TCH = T // P      # 18 t-chunks of 128
# exp tiles in flight: big enough that ScalarE (the bottleneck engine) can
# run well ahead of the AV consumers while the q/v branches finish on the PE
ET_BUFS = 26

_PROGRAM = None
_last_in_maps = None


def _build_program(debug=False):
    """Build the SPMD Bass/Tile program (identical for all 8 cores)."""
    from contextlib import ExitStack

    import concourse.bacc as bacc
    import concourse.mybir as mybir
    import concourse.tile as tile
    from concourse.masks import make_identity
    from concourse.tile_rust import add_dep_helper

    f32 = mybir.dt.float32
    bf16 = mybir.dt.bfloat16
    AF = mybir.ActivationFunctionType

    nc = bacc.Bacc(None, target_bir_lowering=False, debug=False)

    pad_a = nc.declare_dram_parameter("pad_a", [2, P, 2500], bf16, isOutput=False)
    pad_b = nc.declare_dram_parameter("pad_b", [2, P, 2500], bf16, isOutput=False)
    wq = nc.declare_dram_parameter("wq", [2, P, 9 * P], bf16, isOutput=False)
    wk = nc.declare_dram_parameter("wk", [2, P, 9 * P], bf16, isOutput=False)
    wv = nc.declare_dram_parameter("wv", [2, P, 9 * P], bf16, isOutput=False)
    bias_d = nc.declare_dram_parameter("bias", [3, P, 1], f32, isOutput=False)
    # attention part, ^T layout: col = li*256 + lsub*128 + h*32 + c, row = l%128
    out_d = nc.declare_dram_parameter("out", [P, T], f32, isOutput=True)
    qout_d = nc.declare_dram_parameter("qout", [P, T], f32, isOutput=True)

    with tile.TileContext(nc) as tc, ExitStack() as ctx:
        const = ctx.enter_context(tc.tile_pool(name="const", bufs=1))
        sb = ctx.enter_context(tc.tile_pool(name="sb", bufs=1))
        fin = ctx.enter_context(tc.tile_pool(name="fin", bufs=2))

        identity = const.tile([P, P], bf16)
        make_identity(nc, identity)

        # ---- input DMAs (k weights + image B first: they gate phase A) ----
        wk_sb = sb.tile([P, 2 * 9 * P], bf16, tag="wk")
        pb_sb = sb.tile([P, 2 * 2500], bf16, tag="pb")
        wq_sb = sb.tile([P, 2 * 9 * P], bf16, tag="wq")
        pa_sb = sb.tile([P, 2 * 2500], bf16, tag="pa")
        wv_sb = sb.tile([P, 2 * 9 * P], bf16, tag="wv")
        bias_sb = sb.tile([P, 3], f32, tag="bias")
        # first k-tile needs: all of wk + image-B rows 0-11 (cols < 600).
        # Front-load exactly that so the PE starts ~5us earlier.
        for kc in range(2):
            nc.sync.dma_start(wk_sb[:, kc * 1152:(kc + 1) * 1152], wk[kc])
            nc.sync.dma_start(pb_sb[:, kc * 2500:kc * 2500 + 600],
                              pad_b[kc][:, 0:600])
        for kc in range(2):
            nc.sync.dma_start(wq_sb[:, kc * 1152:(kc + 1) * 1152], wq[kc])
            nc.sync.dma_start(pa_sb[:, kc * 2500:kc * 2500 + 600],
                              pad_a[kc][:, 0:600])
        for r in range(3):
            nc.sync.dma_start(bias_sb[:, r:r + 1], bias_d[r])
        for kc in range(2):
            nc.sync.dma_start(pb_sb[:, kc * 2500 + 600:(kc + 1) * 2500],
                              pad_b[kc][:, 600:2500])
            nc.sync.dma_start(pa_sb[:, kc * 2500 + 600:(kc + 1) * 2500],
                              pad_a[kc][:, 600:2500])
        for kc in range(2):
            nc.sync.dma_start(wv_sb[:, kc * 1152:(kc + 1) * 1152], wv[kc])

        kT = sb.tile([P, T], bf16, tag="kT")
        qT = sb.tile([P, T], bf16, tag="qT")
        qTf = sb.tile([P, T], f32, tag="qTf")
        vT = sb.tile([P, T], bf16, tag="vT")
        # block-diagonal q for dense scores: head h lives at rows 32h:32h+32,
        # cols h*T + l; all other rows zero.
        qblk = sb.tile([P, 4 * T], bf16, tag="qblk")
        # v stream for flipped AV: per (chunk j, head h) 33 cols = [v(32)|ones]
        vstr = sb.tile([P, N_TCH * 4 * 33], bf16, tag="vstr")
        outT = sb.tile([P, T], f32, tag="outT")
        ep = ctx.enter_context(tc.tile_pool(name="ep", bufs=2))

        # zero qblk during the input-DMA wait (its off-head rows must be 0)
        nc.vector.memset(qblk[:], 0.0)
        ones_cols = vstr.rearrange("p (b c) -> p b c", c=33)[:, :, 32:33]
        nc.vector.memset(ones_cols, 1.0)

        psum = ctx.enter_context(tc.tile_pool(name="psum", bufs=2, space="PSUM"))

        # ================= Phase A: branch matmuls =================
        def branch_tile(w_sb, img_sb, dest, role, tt, dest2=None):
            (t0, nt, r0, nr) = tt
            ps = psum.tile([P, 1024], f32, tag="sc", bufs=3,
                           name=f"br_{role}_{t0}")
            mm = 0
            for kc in range(2):
                pv = img_sb[:, kc * 2500:(kc + 1) * 2500].rearrange(
                    "p (r c) -> p r c", c=50)
                wv_ = w_sb[:, kc * 1152:(kc + 1) * 1152]
                for di in range(3):
                    for dj in range(3):
                        tap = di * 3 + dj
                        rhs = pv[:, r0 + di:r0 + di + nr, dj:dj + 48]
                        nc.tensor.matmul(
                            ps[:, 0:nt], wv_[:, tap * P:(tap + 1) * P], rhs,
                            start=(mm == 0), stop=(mm == 17))
                        mm += 1
            nc.vector.tensor_scalar_add(dest[:, t0:t0 + nt], ps[:, 0:nt],
                                        bias_sb[:, role:role + 1])
            if dest2 is not None:
                nc.vector.tensor_scalar_add(dest2[:, t0:t0 + nt], ps[:, 0:nt],
                                            bias_sb[:, role:role + 1])

        def qblk_dma(tt):
            (t0, nt, _, _) = tt
            for h in range(4):
                nc.sync.dma_start(
                    qblk[h * 32:(h + 1) * 32, h * T + t0:h * T + t0 + nt],
                    qT[h * 32:(h + 1) * 32, t0:t0 + nt])

        def v_transpose(j):
            tp = psum.tile([P, P], bf16, tag="sc", bufs=3, name=f"tp_{j}")
            nc.tensor.transpose(tp[:], vT[:, j * P:(j + 1) * P], identity[:])
            dst = vstr[:, j * 132:(j + 1) * 132].rearrange(
                "p (h c) -> p h c", c=33)[:, :, 0:32]
            src = tp.rearrange("p (h c) -> p h c", c=32)
            nc.vector.tensor_copy(dst, src)

        # ================= Phase B: attention =================
        sc_tiles = {}
        et_tiles = {}
        qv = qblk.rearrange("p (h l) -> p h l", l=T)

        def scores(li, j):
            l0 = li * NL
            sc = psum.tile([P, 4 * NL], f32, tag="sc", bufs=3,
                           name=f"sc_{li}_{j}")
            kch = kT[:, j * P:(j + 1) * P]
            nc.tensor.matmul(sc[:, 0:512], kch, qv[:, 0:2, l0:l0 + NL],
                             start=True, stop=True)
            last = nc.tensor.matmul(sc[:, 512:1024], kch,
                                    qv[:, 2:4, l0:l0 + NL],
                                    start=True, stop=True)
            sc_tiles[(li, j)] = sc
            return last

        def exp_chunk(li, j):
            sc = sc_tiles.pop((li, j))
            et = ep.tile([P, 4 * NL], bf16, tag="e", bufs=ET_BUFS,
                         name=f"e_{li}_{j}")
            nc.scalar.activation(et[:], sc[:], AF.Exp, scale=-0.0625)
            et_tiles[(li, j)] = et

        def av_chunk(li, j, avp, dep=None):
            et = et_tiles.pop((li, j))
            for h in range(4):
                for ls in range(2):
                    av = nc.tensor.matmul(
                        avp[:, (ls * 4 + h) * 33:(ls * 4 + h) * 33 + 33],
                        et[:, h * NL + ls * P:h * NL + ls * P + P],
                        vstr[:, (j * 4 + h) * 33:(j * 4 + h) * 33 + 33],
                        start=(j == 0), stop=(j == N_TCH - 1),
                        skip_group_check=True)
                    if dep is not None:
                        # keep next scores AHEAD of this exp-gated AV in the
                        # in-order PE stream (scheduling-only ordering edge)
                        add_dep_helper(av.ins, dep.ins,
                                       reason="scores(t+1) before AV(t)")
                        dep = None

        def finalize(li, avp):
            # one strided reciprocal for all 8 denominators (ones cols)
            rc8 = fin.tile([P, 8], f32, tag="recip", bufs=2, name=f"rc_{li}")
            den = avp.rearrange("p (b c) -> p b c", c=33)[:, :, 32:33]
            nc.vector.reciprocal(rc8.rearrange("p (b o) -> p b o", o=1), den)
            for ls in range(2):
                for h in range(4):
                    base = (ls * 4 + h) * 33
                    nc.vector.tensor_scalar_mul(
                        outT[:, li * NL + ls * P + h * 32:
                             li * NL + ls * P + h * 32 + 32],
                        avp[:, base:base + 32],
                        rc8[:, ls * 4 + h:ls * 4 + h + 1])
            nc.sync.dma_start(out_d[:, li * NL:(li + 1) * NL],
                              outT[:, li * NL:(li + 1) * NL])

        # ---------------- schedule ----------------
        # Unified score/AV cursors over flat chunk index c = li*18 + j.
        # Scores for chunk c become available once the k tiles covering t
        # chunk j and the q tiles covering l-tile li are done; they are
        # emitted greedily (up to ET_BUFS ahead of AV) so ScalarE — the
        # critical engine at ~1.1us per exp chunk — starts ~22us in and
        # never starves while the PE finishes the q/v branches.
        NCH = N_LT * N_TCH
        state = {"sc": 0, "av": 0, "khi": -1, "qhi": -1, "dep": None}
        av_tiles = {}

        def emit_scores_avail(max_n=NCH):
            # paced emission: a branch unit is ~3.6us of PE work and ScalarE
            # drains ~1.1us/chunk, so callers cap at ~4 per unit — emitting
            # greedily would head-of-line-block the PE on sc-ring waits
            n = 0
            while (state["sc"] < NCH and n < max_n
                   and (state["sc"] - state["av"]) < ET_BUFS):
                li, j = divmod(state["sc"], N_TCH)
                if li > state["qhi"] or j > state["khi"]:
                    return
                state["dep"] = scores(li, j)
                exp_chunk(li, j)
                state["sc"] += 1
                n += 1

        def emit_av():
            li, j = divmod(state["av"], N_TCH)
            if j == 0:
                av_tiles[li] = psum.tile([P, 264], f32, tag="av", bufs=2,
                                         name=f"avp_{li}")
            av_chunk(li, j, av_tiles[li], dep=state["dep"])
            state["dep"] = None
            state["av"] += 1
            if j == 4 and li > 0:
                finalize(li - 1, av_tiles.pop(li - 1))

        state["tp"] = -1  # highest v transpose emitted

        # Phase A head: k tile 0, then a 288-col q head tile (covers l-tile
        # 0's 256 stream cols) whose qblk DMA latency hides under k tile 1;
        # the first scores issue ~24us in — ScalarE, the critical engine,
        # starts there. Scores for later t-chunks unlock per k tile (khi).
        K_HI = [2, 6, 10, 14, 17]
        branch_tile(wk_sb, pb_sb, kT, 1, T_TILES[0])
        state["khi"] = K_HI[0]
        branch_tile(wq_sb, pa_sb, qT, 0, (0, 288, 0, 6), dest2=qTf)
        qblk_dma((0, 288, 0, 6))
        state["qhi"] = 0
        for ti in range(1, 5):
            branch_tile(wk_sb, pb_sb, kT, 1, T_TILES[ti])
            state["khi"] = K_HI[ti]
            emit_scores_avail(4)

        # Remaining q/v branch work, split into HALF tiles (~1.8us of PE
        # work each) so the sc-ring backlog (bufs=3, ~3.3us of ScalarE
        # runway) covers each injected burst without the exp stream running
        # dry. v halves are injected when the next AV chunk is blocked on a
        # missing transpose; q halves predictively, 8 chunks before the
        # scores cursor would block on l-tile coverage.
        HALF = [(480, 240, 10, 5), (720, 240, 15, 5), (960, 240, 20, 5),
                (1200, 240, 25, 5), (1440, 240, 30, 5), (1680, 240, 35, 5),
                (1920, 192, 40, 4), (2112, 192, 44, 4)]
        v_units = [(tt, (tt[0] + tt[1]) // P - 1) for tt in
                   [(0, 240, 0, 5), (240, 240, 5, 5)] + HALF]
        q_units = [(tt, (tt[0] + tt[1]) // NL - 1) for tt in
                   [(288, 192, 6, 4)] + HALF]

        def inject_v():
            tt, tpmax = v_units.pop(0)
            branch_tile(wv_sb, pb_sb, vT, 2, tt)
            while state["tp"] < tpmax:
                state["tp"] += 1
                v_transpose(state["tp"])

        def inject_q():
            tt, qhi = q_units.pop(0)
            branch_tile(wq_sb, pa_sb, qT, 0, tt, dest2=qTf)
            qblk_dma(tt)
            state["qhi"] = qhi
            if not q_units:
                nc.sync.dma_start(qout_d[:], qTf[:])

        while state["av"] < NCH:
            # scores first, whenever available — ScalarE is the bottleneck
            # and the sc-ring (bufs=3) paces the PE against it; the et ring
            # (26) absorbs the run-ahead while AVs wait on v transposes
            emit_scores_avail(2)
            if q_units and state["sc"] >= (state["qhi"] + 1) * N_TCH - 8:
                inject_q()
            li, j = divmod(state["av"], N_TCH)
            if li == 0 and j > state["tp"] and v_units:
                inject_v()
            elif state["av"] < state["sc"]:
                emit_av()
            elif v_units:
                inject_v()
            elif q_units:
                inject_q()
        finalize(N_LT - 1, av_tiles.pop(N_LT - 1))

        if debug:
            for nm, t in [("dbg_qT", qT), ("dbg_kT", kT), ("dbg_vT", vT),
                          ("dbg_vstr", vstr), ("dbg_qblk", qblk),
                          ("dbg_qTf", qTf)]:
                dd = nc.declare_dram_parameter(nm, list(t.shape), t.dtype,
                                               isOutput=True)
                nc.sync.dma_start(dd[:], t[:])

    nc.compile()
    return nc


def _fold_weights(dw_w, bn_gamma, bn_beta, bn_mean, bn_var, pw_w, pw_b, lin_w):
    """Fold BN + pointwise conv + linear (+ depthwise taps) per branch.

    Returns Wtap [6, 9, 256, 256] (float32) and bias c [6, 256]."""
    dw = dw_w.astype(np.float64)
    g = bn_gamma.astype(np.float64)
    b = bn_beta.astype(np.float64)
    m = bn_mean.astype(np.float64)
    v = bn_var.astype(np.float64)
    pw = pw_w.astype(np.float64)
    pb = pw_b.astype(np.float64)
    lw = lin_w.astype(np.float64)

    scale = g / np.sqrt(v + EPS)
    shift = b - m * scale
    M = np.einsum("noc,ncd->nod", lw, pw)
    W = M * scale[:, None, :]
    c = np.einsum("noc,nc->no", M, shift) + np.einsum("noc,nc->no", lw, pb)
    Wtap = W[:, None, :, :] * dw.transpose(0, 2, 3, 1).reshape(6, 9, 1, 256)
    return Wtap.astype(np.float32), c.astype(np.float32)


def _bf16(a):
    import ml_dtypes
    return a.astype(ml_dtypes.bfloat16)


def _pad_images(x):
    """x [B, T, 256] -> per batch channel-major zero-padded bf16 [2,128,2500]."""
    out = np.zeros((B, 2, P, 50, 50), dtype=np.float32)
    img = np.ascontiguousarray(x.transpose(0, 2, 1)).reshape(B, DIM, HW, HW)
    out[:, :, :, 1:49, 1:49] = img.reshape(B, 2, P, HW, HW)
    return _bf16(out.reshape(B, 2, P, 2500))


def _wtap_lhsT(Wtap, branch, g):
    """Pack lhsT layout [2, 128, 9*128] for a branch restricted to quad g."""
    rows = slice(g * P, (g + 1) * P)
    out = np.empty((2, P, 9 * P), dtype=np.float32)
    for kc in range(2):
        for tap in range(9):
            blk = Wtap[branch, tap][rows, kc * P:(kc + 1) * P]
            out[kc, :, tap * P:(tap + 1) * P] = blk.T
    return _bf16(out)


def kernel(x1, x2, dw_w, bn_gamma, bn_beta, bn_mean, bn_var, pw_w, pw_b, lin_w,
           h1=HW, w1=HW, h2=HW, w2=HW):
    global _PROGRAM
    from concourse.bass_utils import run_bass_kernel_spmd

    x1 = np.asarray(x1, dtype=np.float32)
    x2 = np.asarray(x2, dtype=np.float32)

    Wtap, c = _fold_weights(np.asarray(dw_w), np.asarray(bn_gamma),
                            np.asarray(bn_beta), np.asarray(bn_mean),
                            np.asarray(bn_var), np.asarray(pw_w),
                            np.asarray(pw_b), np.asarray(lin_w))
    pad1 = _pad_images(x1)
    pad2 = _pad_images(x2)

    if _PROGRAM is None:
        _PROGRAM = _build_program()
    nc = _PROGRAM

    # core layout: core = m*4 + b*2 + g
    in_maps = []
    for m in range(2):
        qbr, kbr, vbr = (0, 4, 5) if m == 0 else (3, 1, 2)
        pa, pb_ = (pad1, pad2) if m == 0 else (pad2, pad1)
        for b in range(2):
            for g in range(2):
                bias = np.stack([c[qbr, g * P:(g + 1) * P],
                                 c[kbr, g * P:(g + 1) * P],
                                 c[vbr, g * P:(g + 1) * P]])[:, :, None]
                in_maps.append({
                    "pad_a": np.ascontiguousarray(pa[b]),
                    "pad_b": np.ascontiguousarray(pb_[b]),
                    "wq": _wtap_lhsT(Wtap, qbr, g),
                    "wk": _wtap_lhsT(Wtap, kbr, g),
                    "wv": _wtap_lhsT(Wtap, vbr, g),
                    "bias": np.ascontiguousarray(bias),
                })

    global _last_in_maps
    _last_in_maps = in_maps
    res = run_bass_kernel_spmd(nc, in_maps, list(range(N_CORES)))

    o = np.empty((2, 2, HEADS, T, DH), dtype=np.float32)
    for m in range(2):
        for b in range(2):
            for g in range(2):
                core = m * 4 + b * 2 + g
                att = res.results[core]["out"]
                qf = res.results[core]["qout"]
                a = att.reshape(P, N_LT, 2, 4, DH).transpose(1, 2, 0, 3, 4)
                a = a.reshape(T, 4, DH)
                qq = qf.reshape(4, DH, T).transpose(2, 0, 1)
                o[m, b, 4 * g:4 * g + 4] = (a + qq).transpose(1, 0, 2)
    o1 = o[0].reshape(B, T, HEADS * DH)
    o2 = o[1].reshape(B, T, HEADS * DH)
    return o1, o2


# revision 20
# speedup vs baseline: 1.0085x; 1.0047x over previous
"""Trainium2 Bass kernel for nn_CViTFlow (cross-attention ViT flow block).

Math (per the module):
  two token streams x1,x2 [B,T,256] viewed as [B,256,48,48] images.
  6 branches (q1,k1,v1,q2,k2,v2): depthwise3x3 -> BN(eval) -> 1x1 conv -> Linear.
  o1 = softmax(-(q1 k2^T / 16)) v2 + q1 ;  o2 = softmax(-(q2 k1^T / 16)) v1 + q2
  both reshaped [B,H,T,DH] -> [B,T,256] with a plain (head-major) reshape.

Kernel strategy (v2 — full-array attention, HAM-warm):
  * Host folds BN + 1x1conv + Linear into one 256x256 matrix W and bias c per
    branch, then folds the depthwise 3x3 into 9 "tap" matrices, so a branch is
    9 shifted matmuls accumulated in PSUM (dense K=128/M=128 on the PE).
  * 8 cores = (map m) x (batch b) x (head-quad g); no collectives.
  * Attention is restructured so every matmul uses the FULL 128x128 array
    (the PE_HAM clock gate throttles the PE to 1.2 GHz when it doesn't see
    dense activity — the previous kernel ran its whole attention phase at
    half clock):
      - scores: stationary = kT 128-chunk [128=(4h x 32d), 128 t] (dense),
        stream = qblk, a block-diagonal copy of qT (head h's rows only in
        block h, zeros elsewhere). Cross-head terms vanish exactly, giving
        sc[t, (h,l)] per-head scores with full-K matmuls.
      - AV is flipped: stationary = exp tile [128 t, 128 l] (dense), stream =
        v chunk [128 t, 32+1 cols] (ones col -> softmax denominator), so the
        big operand rides the weight path (LDW) and the stream is tiny.
        Output lands ^T as [l, (h,c)]; the host un-transposes.
  * exp on ScalarE ([128,1024] per chunk) is the pipeline's critical engine
    (~(1024+352)/1.2 ns each); the branch phase is interleaved with l-tile 0
    of attention so ScalarE starts ~20us earlier.
  * q residual is DMA'd out in f32 and added on the host.
"""

import numpy as np

B = 2
T = 2304
DIM = 256
HEADS = 8
DH = 32
HW = 48
EPS = 1e-5
P = 128
N_CORES = 8

# t-tiles for the branch phase: row-aligned in the 48x48 image (10/8 rows)
T_TILES = [(0, 480, 0, 10), (480, 480, 10, 10), (960, 480, 20, 10),
           (1440, 480, 30, 10), (1920, 384, 40, 8)]
NL = 256            # l-tile width (2 x 128 sub-tiles)
N_LT = T // NL      # 9 l-tiles
N_TCH = T // P      # 18 t-chunks of 128
# exp tiles in flight: big enough that ScalarE (the bottleneck engine) can
# run well ahead of the AV consumers while the q/v branches finish on the PE
ET_BUFS = 26

_PROGRAM = None
_last_in_maps = None


def _build_program(debug=False):
    """Build the SPMD Bass/Tile program (identical for all 8 cores)."""
    from contextlib import ExitStack

    import concourse.bacc as bacc
    import concourse.mybir as mybir
    import concourse.tile as tile
    from concourse.masks import make_identity
    from concourse.tile_rust import add_dep_helper

    f32 = mybir.dt.float32
    bf16 = mybir.dt.bfloat16
    AF = mybir.ActivationFunctionType

    nc = bacc.Bacc(None, target_bir_lowering=False, debug=False)

    pad_a = nc.declare_dram_parameter("pad_a", [2, P, 2500], bf16, isOutput=False)
    pad_b = nc.declare_dram_parameter("pad_b", [2, P, 2500], bf16, isOutput=False)
    wq = nc.declare_dram_parameter("wq", [2, P, 9 * P], bf16, isOutput=False)
    wk = nc.declare_dram_parameter("wk", [2, P, 9 * P], bf16, isOutput=False)
    wv = nc.declare_dram_parameter("wv", [2, P, 9 * P], bf16, isOutput=False)
    bias_d = nc.declare_dram_parameter("bias", [3, P, 1], f32, isOutput=False)
    # attention part, ^T layout: col = li*256 + lsub*128 + h*32 + c, row = l%128
    out_d = nc.declare_dram_parameter("out", [P, T], f32, isOutput=True)
    qout_d = nc.declare_dram_parameter("qout", [P, T], f32, isOutput=True)

    with tile.TileContext(nc) as tc, ExitStack() as ctx:
        const = ctx.enter_context(tc.tile_pool(name="const", bufs=1))
        sb = ctx.enter_context(tc.tile_pool(name="sb", bufs=1))
        fin = ctx.enter_context(tc.tile_pool(name="fin", bufs=2))

        identity = const.tile([P, P], bf16)
        make_identity(nc, identity)

        # ---- input DMAs (k weights + image B first: they gate phase A) ----
        wk_sb = sb.tile([P, 2 * 9 * P], bf16, tag="wk")
        pb_sb = sb.tile([P, 2 * 2500], bf16, tag="pb")
        wq_sb = sb.tile([P, 2 * 9 * P], bf16, tag="wq")
        pa_sb = sb.tile([P, 2 * 2500], bf16, tag="pa")
        wv_sb = sb.tile([P, 2 * 9 * P], bf16, tag="wv")
        bias_sb = sb.tile([P, 3], f32, tag="bias")
        # first k-tile needs: all of wk + image-B rows 0-11 (cols < 600).
        # Front-load exactly that so the PE starts ~5us earlier.
        for kc in range(2):
            nc.sync.dma_start(wk_sb[:, kc * 1152:(kc + 1) * 1152], wk[kc])
            nc.sync.dma_start(pb_sb[:, kc * 2500:kc * 2500 + 600],
                              pad_b[kc][:, 0:600])
        for r in range(3):
            nc.sync.dma_start(bias_sb[:, r:r + 1], bias_d[r])
        for kc in range(2):
            nc.sync.dma_start(pb_sb[:, kc * 2500 + 600:(kc + 1) * 2500],
                              pad_b[kc][:, 600:2500])
        for kc in range(2):
            nc.sync.dma_start(wq_sb[:, kc * 1152:(kc + 1) * 1152], wq[kc])
            nc.sync.dma_start(pa_sb[:, kc * 2500:(kc + 1) * 2500], pad_a[kc])
        for kc in range(2):
            nc.sync.dma_start(wv_sb[:, kc * 1152:(kc + 1) * 1152], wv[kc])

        kT = sb.tile([P, T], bf16, tag="kT")
        qT = sb.tile([P, T], bf16, tag="qT")
        qTf = sb.tile([P, T], f32, tag="qTf")
        vT = sb.tile([P, T], bf16, tag="vT")
        # block-diagonal q for dense scores: head h lives at rows 32h:32h+32,
        # cols h*T + l; all other rows zero.
        qblk = sb.tile([P, 4 * T], bf16, tag="qblk")
        # v stream for flipped AV: per (chunk j, head h) 33 cols = [v(32)|ones]
        vstr = sb.tile([P, N_TCH * 4 * 33], bf16, tag="vstr")
        outT = sb.tile([P, T], f32, tag="outT")
        ep = ctx.enter_context(tc.tile_pool(name="ep", bufs=2))

        # zero qblk during the input-DMA wait (its off-head rows must be 0)
        nc.vector.memset(qblk[:], 0.0)
        ones_cols = vstr.rearrange("p (b c) -> p b c", c=33)[:, :, 32:33]
        nc.vector.memset(ones_cols, 1.0)

        psum = ctx.enter_context(tc.tile_pool(name="psum", bufs=2, space="PSUM"))

        # ================= Phase A: branch matmuls =================
        def branch_tile(w_sb, img_sb, dest, role, tt, dest2=None):
            (t0, nt, r0, nr) = tt
            ps = psum.tile([P, 1024], f32, tag="sc", bufs=3,
                           name=f"br_{role}_{t0}")
            mm = 0
            for kc in range(2):
                pv = img_sb[:, kc * 2500:(kc + 1) * 2500].rearrange(
                    "p (r c) -> p r c", c=50)
                wv_ = w_sb[:, kc * 1152:(kc + 1) * 1152]
                for di in range(3):
                    for dj in range(3):
                        tap = di * 3 + dj
                        rhs = pv[:, r0 + di:r0 + di + nr, dj:dj + 48]
                        nc.tensor.matmul(
                            ps[:, 0:nt], wv_[:, tap * P:(tap + 1) * P], rhs,
                            start=(mm == 0), stop=(mm == 17))
                        mm += 1
            nc.vector.tensor_scalar_add(dest[:, t0:t0 + nt], ps[:, 0:nt],
                                        bias_sb[:, role:role + 1])
            if dest2 is not None:
                nc.vector.tensor_scalar_add(dest2[:, t0:t0 + nt], ps[:, 0:nt],
                                            bias_sb[:, role:role + 1])

        def qblk_dma(tt):
            (t0, nt, _, _) = tt
            for h in range(4):
                nc.sync.dma_start(
                    qblk[h * 32:(h + 1) * 32, h * T + t0:h * T + t0 + nt],
                    qT[h * 32:(h + 1) * 32, t0:t0 + nt])

        def v_transpose(j):
            tp = psum.tile([P, P], bf16, tag="sc", bufs=3, name=f"tp_{j}")
            nc.tensor.transpose(tp[:], vT[:, j * P:(j + 1) * P], identity[:])
            dst = vstr[:, j * 132:(j + 1) * 132].rearrange(
                "p (h c) -> p h c", c=33)[:, :, 0:32]
            src = tp.rearrange("p (h c) -> p h c", c=32)
            nc.vector.tensor_copy(dst, src)

        # ================= Phase B: attention =================
        sc_tiles = {}
        et_tiles = {}
        qv = qblk.rearrange("p (h l) -> p h l", l=T)

        def scores(li, j):
            l0 = li * NL
            sc = psum.tile([P, 4 * NL], f32, tag="sc", bufs=3,
                           name=f"sc_{li}_{j}")
            kch = kT[:, j * P:(j + 1) * P]
            nc.tensor.matmul(sc[:, 0:512], kch, qv[:, 0:2, l0:l0 + NL],
                             start=True, stop=True)
            last = nc.tensor.matmul(sc[:, 512:1024], kch,
                                    qv[:, 2:4, l0:l0 + NL],
                                    start=True, stop=True)
            sc_tiles[(li, j)] = sc
            return last

        def exp_chunk(li, j):
            sc = sc_tiles.pop((li, j))
            et = ep.tile([P, 4 * NL], bf16, tag="e", bufs=ET_BUFS,
                         name=f"e_{li}_{j}")
            nc.scalar.activation(et[:], sc[:], AF.Exp, scale=-0.0625)
            et_tiles[(li, j)] = et

        def av_chunk(li, j, avp, dep=None):
            et = et_tiles.pop((li, j))
            for h in range(4):
                for ls in range(2):
                    av = nc.tensor.matmul(
                        avp[:, (ls * 4 + h) * 33:(ls * 4 + h) * 33 + 33],
                        et[:, h * NL + ls * P:h * NL + ls * P + P],
                        vstr[:, (j * 4 + h) * 33:(j * 4 + h) * 33 + 33],
                        start=(j == 0), stop=(j == N_TCH - 1),
                        skip_group_check=True)
                    if dep is not None:
                        # keep next scores AHEAD of this exp-gated AV in the
                        # in-order PE stream (scheduling-only ordering edge)
                        add_dep_helper(av.ins, dep.ins,
                                       reason="scores(t+1) before AV(t)")
                        dep = None

        def finalize(li, avp):
            # one strided reciprocal for all 8 denominators (ones cols)
            rc8 = fin.tile([P, 8], f32, tag="recip", bufs=2, name=f"rc_{li}")
            den = avp.rearrange("p (b c) -> p b c", c=33)[:, :, 32:33]
            nc.vector.reciprocal(rc8.rearrange("p (b o) -> p b o", o=1), den)
            for ls in range(2):
                for h in range(4):
                    base = (ls * 4 + h) * 33
                    nc.vector.tensor_scalar_mul(
                        outT[:, li * NL + ls * P + h * 32:
                             li * NL + ls * P + h * 32 + 32],
                        avp[:, base:base + 32],
                        rc8[:, ls * 4 + h:ls * 4 + h + 1])
            nc.sync.dma_start(out_d[:, li * NL:(li + 1) * NL],
                              outT[:, li * NL:(li + 1) * NL])

        # ---------------- schedule ----------------
        # Unified score/AV cursors over flat chunk index c = li*18 + j.
        # Scores for chunk c become available once the k tiles covering t
        # chunk j and the q tiles covering l-tile li are done; they are
        # emitted greedily (up to ET_BUFS ahead of AV) so ScalarE — the
        # critical engine at ~1.1us per exp chunk — starts ~22us in and
        # never starves while the PE finishes the q/v branches.
        NCH = N_LT * N_TCH
        state = {"sc": 0, "av": 0, "khi": -1, "qhi": -1, "dep": None}
        av_tiles = {}

        def emit_scores_avail(max_n=NCH):
            # paced emission: a branch unit is ~3.6us of PE work and ScalarE
            # drains ~1.1us/chunk, so callers cap at ~4 per unit — emitting
            # greedily would head-of-line-block the PE on sc-ring waits
            n = 0
            while (state["sc"] < NCH and n < max_n
                   and (state["sc"] - state["av"]) < ET_BUFS):
                li, j = divmod(state["sc"], N_TCH)
                if li > state["qhi"] or j > state["khi"]:
                    return
                state["dep"] = scores(li, j)
                exp_chunk(li, j)
                state["sc"] += 1
                n += 1

        def emit_av():
            li, j = divmod(state["av"], N_TCH)
            if j == 0:
                av_tiles[li] = psum.tile([P, 264], f32, tag="av", bufs=2,
                                         name=f"avp_{li}")
            av_chunk(li, j, av_tiles[li], dep=state["dep"])
            state["dep"] = None
            state["av"] += 1
            if j == 4 and li > 0:
                finalize(li - 1, av_tiles.pop(li - 1))

        state["tp"] = -1  # highest v transpose emitted

        # Phase A head: k tile 0, then a 288-col q head tile (covers l-tile
        # 0's 256 stream cols) whose qblk DMA latency hides under k tile 1;
        # the first scores issue ~24us in — ScalarE, the critical engine,
        # starts there. Scores for later t-chunks unlock per k tile (khi).
        K_HI = [2, 6, 10, 14, 17]
        branch_tile(wk_sb, pb_sb, kT, 1, T_TILES[0])
        state["khi"] = K_HI[0]
        branch_tile(wq_sb, pa_sb, qT, 0, (0, 288, 0, 6), dest2=qTf)
        qblk_dma((0, 288, 0, 6))
        state["qhi"] = 0
        for ti in range(1, 5):
            branch_tile(wk_sb, pb_sb, kT, 1, T_TILES[ti])
            state["khi"] = K_HI[ti]
            emit_scores_avail(3)

        # Remaining q/v branch work, split into HALF tiles (~1.8us of PE
        # work each) so the sc-ring backlog (bufs=3, ~3.3us of ScalarE
        # runway) covers each injected burst without the exp stream running
        # dry. v halves are injected when the next AV chunk is blocked on a
        # missing transpose; q halves predictively, 8 chunks before the
        # scores cursor would block on l-tile coverage.
        HALF = [(480, 240, 10, 5), (720, 240, 15, 5), (960, 240, 20, 5),
                (1200, 240, 25, 5), (1440, 240, 30, 5), (1680, 240, 35, 5),
                (1920, 192, 40, 4), (2112, 192, 44, 4)]
        v_units = [(tt, (tt[0] + tt[1]) // P - 1) for tt in
                   [(0, 240, 0, 5), (240, 240, 5, 5)] + HALF]
        q_units = [(tt, (tt[0] + tt[1]) // NL - 1) for tt in
                   [(288, 192, 6, 4)] + HALF]

        def inject_v():
            tt, tpmax = v_units.pop(0)
            branch_tile(wv_sb, pb_sb, vT, 2, tt)
            while state["tp"] < tpmax:
                state["tp"] += 1
                v_transpose(state["tp"])

        def inject_q():
            tt, qhi = q_units.pop(0)
            branch_tile(wq_sb, pa_sb, qT, 0, tt, dest2=qTf)
            qblk_dma(tt)
            state["qhi"] = qhi
            if not q_units:
                nc.sync.dma_start(qout_d[:], qTf[:])

        while state["av"] < NCH:
            # scores first, whenever available — ScalarE is the bottleneck
            # and the sc-ring (bufs=3) paces the PE against it; the et ring
            # (26) absorbs the run-ahead while AVs wait on v transposes
            emit_scores_avail(2)
            if q_units and state["sc"] >= (state["qhi"] + 1) * N_TCH - 8:
                inject_q()
            li, j = divmod(state["av"], N_TCH)
            if li == 0 and j > state["tp"] and v_units:
                inject_v()
            elif state["av"] < state["sc"]:
                emit_av()
            elif v_units:
                inject_v()
            elif q_units:
                inject_q()
        finalize(N_LT - 1, av_tiles.pop(N_LT - 1))

        if debug:
            for nm, t in [("dbg_qT", qT), ("dbg_kT", kT), ("dbg_vT", vT),
                          ("dbg_vstr", vstr), ("dbg_qblk", qblk),
                          ("dbg_qTf", qTf)]:
                dd = nc.declare_dram_parameter(nm, list(t.shape), t.dtype,
                                               isOutput=True)
                nc.sync.dma_start(dd[:], t[:])

    nc.compile()
    return nc


def _fold_weights(dw_w, bn_gamma, bn_beta, bn_mean, bn_var, pw_w, pw_b, lin_w):
    """Fold BN + pointwise conv + linear (+ depthwise taps) per branch.

    Returns Wtap [6, 9, 256, 256] (float32) and bias c [6, 256]."""
    dw = dw_w.astype(np.float64)
    g = bn_gamma.astype(np.float64)
    b = bn_beta.astype(np.float64)
    m = bn_mean.astype(np.float64)
    v = bn_var.astype(np.float64)
    pw = pw_w.astype(np.float64)
    pb = pw_b.astype(np.float64)
    lw = lin_w.astype(np.float64)

    scale = g / np.sqrt(v + EPS)
    shift = b - m * scale
    M = np.einsum("noc,ncd->nod", lw, pw)
    W = M * scale[:, None, :]
    c = np.einsum("noc,nc->no", M, shift) + np.einsum("noc,nc->no", lw, pb)
    Wtap = W[:, None, :, :] * dw.transpose(0, 2, 3, 1).reshape(6, 9, 1, 256)
    return Wtap.astype(np.float32), c.astype(np.float32)


def _bf16(a):
    import ml_dtypes
    return a.astype(ml_dtypes.bfloat16)


def _pad_images(x):
    """x [B, T, 256] -> per batch channel-major zero-padded bf16 [2,128,2500]."""
    out = np.zeros((B, 2, P, 50, 50), dtype=np.float32)
    img = np.ascontiguousarray(x.transpose(0, 2, 1)).reshape(B, DIM, HW, HW)
    out[:, :, :, 1:49, 1:49] = img.reshape(B, 2, P, HW, HW)
    return _bf16(out.reshape(B, 2, P, 2500))


def _wtap_lhsT(Wtap, branch, g):
    """Pack lhsT layout [2, 128, 9*128] for a branch restricted to quad g."""
    rows = slice(g * P, (g + 1) * P)
    out = np.empty((2, P, 9 * P), dtype=np.float32)
    for kc in range(2):
        for tap in range(9):
            blk = Wtap[branch, tap][rows, kc * P:(kc + 1) * P]
            out[kc, :, tap * P:(tap + 1) * P] = blk.T
    return _bf16(out)


def kernel(x1, x2, dw_w, bn_gamma, bn_beta, bn_mean, bn_var, pw_w, pw_b, lin_w,
           h1=HW, w1=HW, h2=HW, w2=HW):
    global _PROGRAM
    from concourse.bass_utils import run_bass_kernel_spmd

    x1 = np.asarray(x1, dtype=np.float32)
    x2 = np.asarray(x2, dtype=np.float32)

    Wtap, c = _fold_weights(np.asarray(dw_w), np.asarray(bn_gamma),
                            np.asarray(bn_beta), np.asarray(bn_mean),
                            np.asarray(bn_var), np.asarray(pw_w),
                            np.asarray(pw_b), np.asarray(lin_w))
    pad1 = _pad_images(x1)
    pad2 = _pad_images(x2)

    if _PROGRAM is None:
        _PROGRAM = _build_program()
    nc = _PROGRAM

    # core layout: core = m*4 + b*2 + g
    in_maps = []
    for m in range(2):
        qbr, kbr, vbr = (0, 4, 5) if m == 0 else (3, 1, 2)
        pa, pb_ = (pad1, pad2) if m == 0 else (pad2, pad1)
        for b in range(2):
            for g in range(2):
                bias = np.stack([c[qbr, g * P:(g + 1) * P],
                                 c[kbr, g * P:(g + 1) * P],
                                 c[vbr, g * P:(g + 1) * P]])[:, :, None]
                in_maps.append({
                    "pad_a": np.ascontiguousarray(pa[b]),
                    "pad_b": np.ascontiguousarray(pb_[b]),
                    "wq": _wtap_lhsT(Wtap, qbr, g),
                    "wk": _wtap_lhsT(Wtap, kbr, g),
                    "wv": _wtap_lhsT(Wtap, vbr, g),
                    "bias": np.ascontiguousarray(bias),
                })

    global _last_in_maps
    _last_in_maps = in_maps
    res = run_bass_kernel_spmd(nc, in_maps, list(range(N_CORES)))

    o = np.empty((2, 2, HEADS, T, DH), dtype=np.float32)
    for m in range(2):
        for b in range(2):
            for g in range(2):
                core = m * 4 + b * 2 + g
                att = res.results[core]["out"]
                qf = res.results[core]["qout"]
                a = att.reshape(P, N_LT, 2, 4, DH).transpose(1, 2, 0, 3, 4)
                a = a.reshape(T, 4, DH)
                qq = qf.reshape(4, DH, T).transpose(2, 0, 1)
                o[m, b, 4 * g:4 * g + 4] = (a + qq).transpose(1, 0, 2)
    o1 = o[0].reshape(B, T, HEADS * DH)
    o2 = o[1].reshape(B, T, HEADS * DH)
    return o1, o2


# revision 23
# speedup vs baseline: 1.0272x; 1.0186x over previous
"""Trainium2 Bass kernel for nn_CViTFlow (cross-attention ViT flow block).

Math (per the module):
  two token streams x1,x2 [B,T,256] viewed as [B,256,48,48] images.
  6 branches (q1,k1,v1,q2,k2,v2): depthwise3x3 -> BN(eval) -> 1x1 conv -> Linear.
  o1 = softmax(-(q1 k2^T / 16)) v2 + q1 ;  o2 = softmax(-(q2 k1^T / 16)) v1 + q2
  both reshaped [B,H,T,DH] -> [B,T,256] with a plain (head-major) reshape.

Kernel strategy (v2 — full-array attention, HAM-warm):
  * Host folds BN + 1x1conv + Linear into one 256x256 matrix W and bias c per
    branch, then folds the depthwise 3x3 into 9 "tap" matrices, so a branch is
    9 shifted matmuls accumulated in PSUM (dense K=128/M=128 on the PE).
  * 8 cores = (map m) x (batch b) x (head-quad g); no collectives.
  * Attention is restructured so every matmul uses the FULL 128x128 array
    (the PE_HAM clock gate throttles the PE to 1.2 GHz when it doesn't see
    dense activity — the previous kernel ran its whole attention phase at
    half clock):
      - scores: stationary = kT 128-chunk [128=(4h x 32d), 128 t] (dense),
        stream = qblk, a block-diagonal copy of qT (head h's rows only in
        block h, zeros elsewhere). Cross-head terms vanish exactly, giving
        sc[t, (h,l)] per-head scores with full-K matmuls.
      - AV is flipped: stationary = exp tile [128 t, 128 l] (dense), stream =
        v chunk [128 t, 32+1 cols] (ones col -> softmax denominator), so the
        big operand rides the weight path (LDW) and the stream is tiny.
        Output lands ^T as [l, (h,c)]; the host un-transposes.
  * exp on ScalarE ([128,1024] per chunk) is the pipeline's critical engine
    (~(1024+352)/1.2 ns each); the branch phase is interleaved with l-tile 0
    of attention so ScalarE starts ~20us earlier.
  * q residual is DMA'd out in f32 and added on the host.
"""

import numpy as np

B = 2
T = 2304
DIM = 256
HEADS = 8
DH = 32
HW = 48
EPS = 1e-5
P = 128
N_CORES = 8

# t-tiles for the branch phase: row-aligned in the 48x48 image (10/8 rows)
T_TILES = [(0, 480, 0, 10), (480, 480, 10, 10), (960, 480, 20, 10),
           (1440, 480, 30, 10), (1920, 384, 40, 8)]
NL = 256            # l-tile width (2 x 128 sub-tiles)
N_LT = T // NL      # 9 l-tiles
N_TCH = T // P      # 18 t-chunks of 128
# exp tiles in flight: big enough that ScalarE (the bottleneck engine) can
# run well ahead of the AV consumers while the q/v branches finish on the PE
ET_BUFS = 26

_PROGRAM = None
_last_in_maps = None


def _build_program(debug=False):
    """Build the SPMD Bass/Tile program (identical for all 8 cores)."""
    from contextlib import ExitStack

    import concourse.bacc as bacc
    import concourse.mybir as mybir
    import concourse.tile as tile
    from concourse.masks import make_identity
    from concourse.tile_rust import add_dep_helper

    f32 = mybir.dt.float32
    bf16 = mybir.dt.bfloat16
    AF = mybir.ActivationFunctionType

    nc = bacc.Bacc(None, target_bir_lowering=False, debug=False)

    pad_a = nc.declare_dram_parameter("pad_a", [2, P, 2500], bf16, isOutput=False)
    pad_b = nc.declare_dram_parameter("pad_b", [2, P, 2500], bf16, isOutput=False)
    wq = nc.declare_dram_parameter("wq", [2, P, 9 * P], bf16, isOutput=False)
    wk = nc.declare_dram_parameter("wk", [2, P, 9 * P], bf16, isOutput=False)
    wv = nc.declare_dram_parameter("wv", [2, P, 9 * P], bf16, isOutput=False)
    bias_d = nc.declare_dram_parameter("bias", [3, P, 1], f32, isOutput=False)
    # attention part, ^T layout: col = li*256 + lsub*128 + h*32 + c, row = l%128
    out_d = nc.declare_dram_parameter("out", [P, T], f32, isOutput=True)
    qout_d = nc.declare_dram_parameter("qout", [P, T], f32, isOutput=True)
    avlast_d = nc.declare_dram_parameter("avlast", [P, 264], f32, isOutput=True)

    with tile.TileContext(nc) as tc, ExitStack() as ctx:
        const = ctx.enter_context(tc.tile_pool(name="const", bufs=1))
        sb = ctx.enter_context(tc.tile_pool(name="sb", bufs=1))
        fin = ctx.enter_context(tc.tile_pool(name="fin", bufs=2))

        identity = const.tile([P, P], bf16)
        make_identity(nc, identity)

        # ---- input DMAs (k weights + image B first: they gate phase A) ----
        wk_sb = sb.tile([P, 2 * 9 * P], bf16, tag="wk")
        pb_sb = sb.tile([P, 2 * 2500], bf16, tag="pb")
        wq_sb = sb.tile([P, 2 * 9 * P], bf16, tag="wq")
        pa_sb = sb.tile([P, 2 * 2500], bf16, tag="pa")
        wv_sb = sb.tile([P, 2 * 9 * P], bf16, tag="wv")
        bias_sb = sb.tile([P, 3], f32, tag="bias")
        # first k-tile needs: all of wk + image-B rows 0-11 (cols < 600).
        # Front-load exactly that so the PE starts ~5us earlier.
        for kc in range(2):
            nc.sync.dma_start(wk_sb[:, kc * 1152:(kc + 1) * 1152], wk[kc])
            nc.sync.dma_start(pb_sb[:, kc * 2500:kc * 2500 + 600],
                              pad_b[kc][:, 0:600])
        for kc in range(2):
            nc.sync.dma_start(wq_sb[:, kc * 1152:(kc + 1) * 1152], wq[kc])
            nc.sync.dma_start(pa_sb[:, kc * 2500:kc * 2500 + 600],
                              pad_a[kc][:, 0:600])
        for r in range(3):
            nc.sync.dma_start(bias_sb[:, r:r + 1], bias_d[r])
        for kc in range(2):
            nc.sync.dma_start(pb_sb[:, kc * 2500 + 600:(kc + 1) * 2500],
                              pad_b[kc][:, 600:2500])
            nc.sync.dma_start(pa_sb[:, kc * 2500 + 600:(kc + 1) * 2500],
                              pad_a[kc][:, 600:2500])
        for kc in range(2):
            nc.sync.dma_start(wv_sb[:, kc * 1152:(kc + 1) * 1152], wv[kc])

        kT = sb.tile([P, T], bf16, tag="kT")
        qT = sb.tile([P, T], bf16, tag="qT")
        qTf = sb.tile([P, T], f32, tag="qTf")
        vT = sb.tile([P, T], bf16, tag="vT")
        # block-diagonal q for dense scores: head h lives at rows 32h:32h+32,
        # cols h*T + l; all other rows zero.
        qblk = sb.tile([P, 4 * T], bf16, tag="qblk")
        # v stream for flipped AV: per (chunk j, head h) 33 cols = [v(32)|ones]
        vstr = sb.tile([P, N_TCH * 4 * 33], bf16, tag="vstr")
        outT = sb.tile([P, T], f32, tag="outT")
        ep = ctx.enter_context(tc.tile_pool(name="ep", bufs=2))

        # zero qblk during the input-DMA wait (its off-head rows must be 0)
        nc.vector.memset(qblk[:], 0.0)
        ones_cols = vstr.rearrange("p (b c) -> p b c", c=33)[:, :, 32:33]
        nc.vector.memset(ones_cols, 1.0)

        psum = ctx.enter_context(tc.tile_pool(name="psum", bufs=2, space="PSUM"))

        # ================= Phase A: branch matmuls =================
        def branch_tile(w_sb, img_sb, dest, role, tt, dest2=None):
            (t0, nt, r0, nr) = tt
            ps = psum.tile([P, 1024], f32, tag="sc", bufs=3,
                           name=f"br_{role}_{t0}")
            mm = 0
            for kc in range(2):
                pv = img_sb[:, kc * 2500:(kc + 1) * 2500].rearrange(
                    "p (r c) -> p r c", c=50)
                wv_ = w_sb[:, kc * 1152:(kc + 1) * 1152]
                for di in range(3):
                    for dj in range(3):
                        tap = di * 3 + dj
                        rhs = pv[:, r0 + di:r0 + di + nr, dj:dj + 48]
                        nc.tensor.matmul(
                            ps[:, 0:nt], wv_[:, tap * P:(tap + 1) * P], rhs,
                            start=(mm == 0), stop=(mm == 17))
                        mm += 1
            nc.vector.tensor_scalar_add(dest[:, t0:t0 + nt], ps[:, 0:nt],
                                        bias_sb[:, role:role + 1])
            if dest2 is not None:
                nc.vector.tensor_scalar_add(dest2[:, t0:t0 + nt], ps[:, 0:nt],
                                            bias_sb[:, role:role + 1])

        def qblk_dma(tt):
            (t0, nt, _, _) = tt
            for h in range(4):
                nc.sync.dma_start(
                    qblk[h * 32:(h + 1) * 32, h * T + t0:h * T + t0 + nt],
                    qT[h * 32:(h + 1) * 32, t0:t0 + nt])

        def v_transpose(j):
            tp = psum.tile([P, P], bf16, tag="sc", bufs=3, name=f"tp_{j}")
            nc.tensor.transpose(tp[:], vT[:, j * P:(j + 1) * P], identity[:])
            dst = vstr[:, j * 132:(j + 1) * 132].rearrange(
                "p (h c) -> p h c", c=33)[:, :, 0:32]
            src = tp.rearrange("p (h c) -> p h c", c=32)
            nc.vector.tensor_copy(dst, src)

        # ================= Phase B: attention =================
        sc_tiles = {}
        et_tiles = {}
        qv = qblk.rearrange("p (h l) -> p h l", l=T)

        def scores(li, j):
            l0 = li * NL
            sc = psum.tile([P, 4 * NL], f32, tag="sc", bufs=3,
                           name=f"sc_{li}_{j}")
            kch = kT[:, j * P:(j + 1) * P]
            nc.tensor.matmul(sc[:, 0:512], kch, qv[:, 0:2, l0:l0 + NL],
                             start=True, stop=True)
            last = nc.tensor.matmul(sc[:, 512:1024], kch,
                                    qv[:, 2:4, l0:l0 + NL],
                                    start=True, stop=True)
            sc_tiles[(li, j)] = sc
            return last

        def exp_chunk(li, j):
            sc = sc_tiles.pop((li, j))
            et = ep.tile([P, 4 * NL], bf16, tag="e", bufs=ET_BUFS,
                         name=f"e_{li}_{j}")
            nc.scalar.activation(et[:], sc[:], AF.Exp, scale=-0.0625)
            et_tiles[(li, j)] = et

        def av_chunk(li, j, avp, dep=None):
            et = et_tiles.pop((li, j))
            for h in range(4):
                for ls in range(2):
                    av = nc.tensor.matmul(
                        avp[:, (ls * 4 + h) * 33:(ls * 4 + h) * 33 + 33],
                        et[:, h * NL + ls * P:h * NL + ls * P + P],
                        vstr[:, (j * 4 + h) * 33:(j * 4 + h) * 33 + 33],
                        start=(j == 0), stop=(j == N_TCH - 1),
                        skip_group_check=True)
                    if dep is not None:
                        # keep next scores AHEAD of this exp-gated AV in the
                        # in-order PE stream (scheduling-only ordering edge)
                        add_dep_helper(av.ins, dep.ins,
                                       reason="scores(t+1) before AV(t)")
                        dep = None

        def finalize(li, avp):
            # one strided reciprocal for all 8 denominators (ones cols)
            rc8 = fin.tile([P, 8], f32, tag="recip", bufs=2, name=f"rc_{li}")
            den = avp.rearrange("p (b c) -> p b c", c=33)[:, :, 32:33]
            nc.vector.reciprocal(rc8.rearrange("p (b o) -> p b o", o=1), den)
            for ls in range(2):
                for h in range(4):
                    base = (ls * 4 + h) * 33
                    nc.vector.tensor_scalar_mul(
                        outT[:, li * NL + ls * P + h * 32:
                             li * NL + ls * P + h * 32 + 32],
                        avp[:, base:base + 32],
                        rc8[:, ls * 4 + h:ls * 4 + h + 1])
            nc.sync.dma_start(out_d[:, li * NL:(li + 1) * NL],
                              outT[:, li * NL:(li + 1) * NL])

        # ---------------- schedule ----------------
        # Unified score/AV cursors over flat chunk index c = li*18 + j.
        # Scores for chunk c become available once the k tiles covering t
        # chunk j and the q tiles covering l-tile li are done; they are
        # emitted greedily (up to ET_BUFS ahead of AV) so ScalarE — the
        # critical engine at ~1.1us per exp chunk — starts ~22us in and
        # never starves while the PE finishes the q/v branches.
        NCH = N_LT * N_TCH
        state = {"sc": 0, "av": 0, "khi": -1, "qhi": -1, "dep": None}
        av_tiles = {}

        def emit_scores_avail(max_n=NCH):
            # paced emission: a branch unit is ~3.6us of PE work and ScalarE
            # drains ~1.1us/chunk, so callers cap at ~4 per unit — emitting
            # greedily would head-of-line-block the PE on sc-ring waits
            n = 0
            while (state["sc"] < NCH and n < max_n
                   and (state["sc"] - state["av"]) < ET_BUFS):
                li, j = divmod(state["sc"], N_TCH)
                if li > state["qhi"] or j > state["khi"]:
                    return
                state["dep"] = scores(li, j)
                exp_chunk(li, j)
                state["sc"] += 1
                n += 1

        def emit_av():
            li, j = divmod(state["av"], N_TCH)
            if j == 0:
                av_tiles[li] = psum.tile([P, 264], f32, tag="av", bufs=2,
                                         name=f"avp_{li}")
            av_chunk(li, j, av_tiles[li], dep=state["dep"])
            state["dep"] = None
            state["av"] += 1
            if j == 4 and li > 0:
                finalize(li - 1, av_tiles.pop(li - 1))

        state["tp"] = -1  # highest v transpose emitted

        # Phase A head: k tile 0, then a 288-col q head tile (covers l-tile
        # 0's 256 stream cols) whose qblk DMA latency hides under k tile 1;
        # the first scores issue ~24us in — ScalarE, the critical engine,
        # starts there. Scores for later t-chunks unlock per k tile (khi).
        K_HI = [2, 6, 10, 14, 17]
        branch_tile(wk_sb, pb_sb, kT, 1, T_TILES[0])
        state["khi"] = K_HI[0]
        branch_tile(wq_sb, pa_sb, qT, 0, (0, 288, 0, 6), dest2=qTf)
        qblk_dma((0, 288, 0, 6))
        state["qhi"] = 0
        for ti in range(1, 5):
            branch_tile(wk_sb, pb_sb, kT, 1, T_TILES[ti])
            state["khi"] = K_HI[ti]
            emit_scores_avail(3)

        # Remaining q/v branch work, split into HALF tiles (~1.8us of PE
        # work each) so the sc-ring backlog (bufs=3, ~3.3us of ScalarE
        # runway) covers each injected burst without the exp stream running
        # dry. v halves are injected when the next AV chunk is blocked on a
        # missing transpose; q halves predictively, 8 chunks before the
        # scores cursor would block on l-tile coverage.
        HALF = [(480, 240, 10, 5), (720, 240, 15, 5), (960, 240, 20, 5),
                (1200, 240, 25, 5), (1440, 240, 30, 5), (1680, 240, 35, 5),
                (1920, 192, 40, 4), (2112, 192, 44, 4)]
        v_units = [(tt, (tt[0] + tt[1]) // P - 1) for tt in
                   [(0, 240, 0, 5), (240, 240, 5, 5)] + HALF]
        q_units = [(tt, (tt[0] + tt[1]) // NL - 1) for tt in
                   [(288, 192, 6, 4)] + HALF]

        def inject_v():
            tt, tpmax = v_units.pop(0)
            branch_tile(wv_sb, pb_sb, vT, 2, tt)
            while state["tp"] < tpmax:
                state["tp"] += 1
                v_transpose(state["tp"])

        def inject_q():
            tt, qhi = q_units.pop(0)
            branch_tile(wq_sb, pa_sb, qT, 0, tt, dest2=qTf)
            qblk_dma(tt)
            state["qhi"] = qhi
            if not q_units:
                nc.sync.dma_start(qout_d[:], qTf[:])

        while state["av"] < NCH:
            # scores first, whenever available — ScalarE is the bottleneck
            # and the sc-ring (bufs=3) paces the PE against it; the et ring
            # (26) absorbs the run-ahead while AVs wait on v transposes
            emit_scores_avail(2)
            if q_units and state["sc"] >= (state["qhi"] + 1) * N_TCH - 8:
                inject_q()
            li, j = divmod(state["av"], N_TCH)
            if li == 0 and j > state["tp"] and v_units:
                inject_v()
            elif state["av"] < state["sc"]:
                emit_av()
            elif v_units:
                inject_v()
            elif q_units:
                inject_q()
        avl_sb = sb.tile([P, 264], f32, tag="avlast")
        nc.vector.tensor_copy(avl_sb[:], av_tiles.pop(N_LT - 1)[:])
        nc.sync.dma_start(avlast_d[:], avl_sb[:])

        if debug:
            for nm, t in [("dbg_qT", qT), ("dbg_kT", kT), ("dbg_vT", vT),
                          ("dbg_vstr", vstr), ("dbg_qblk", qblk),
                          ("dbg_qTf", qTf)]:
                dd = nc.declare_dram_parameter(nm, list(t.shape), t.dtype,
                                               isOutput=True)
                nc.sync.dma_start(dd[:], t[:])

    nc.compile()
    return nc


def _fold_weights(dw_w, bn_gamma, bn_beta, bn_mean, bn_var, pw_w, pw_b, lin_w):
    """Fold BN + pointwise conv + linear (+ depthwise taps) per branch.

    Returns Wtap [6, 9, 256, 256] (float32) and bias c [6, 256]."""
    dw = dw_w.astype(np.float64)
    g = bn_gamma.astype(np.float64)
    b = bn_beta.astype(np.float64)
    m = bn_mean.astype(np.float64)
    v = bn_var.astype(np.float64)
    pw = pw_w.astype(np.float64)
    pb = pw_b.astype(np.float64)
    lw = lin_w.astype(np.float64)

    scale = g / np.sqrt(v + EPS)
    shift = b - m * scale
    M = np.einsum("noc,ncd->nod", lw, pw)
    W = M * scale[:, None, :]
    c = np.einsum("noc,nc->no", M, shift) + np.einsum("noc,nc->no", lw, pb)
    Wtap = W[:, None, :, :] * dw.transpose(0, 2, 3, 1).reshape(6, 9, 1, 256)
    return Wtap.astype(np.float32), c.astype(np.float32)


def _bf16(a):
    import ml_dtypes
    return a.astype(ml_dtypes.bfloat16)


def _pad_images(x):
    """x [B, T, 256] -> per batch channel-major zero-padded bf16 [2,128,2500]."""
    out = np.zeros((B, 2, P, 50, 50), dtype=np.float32)
    img = np.ascontiguousarray(x.transpose(0, 2, 1)).reshape(B, DIM, HW, HW)
    out[:, :, :, 1:49, 1:49] = img.reshape(B, 2, P, HW, HW)
    return _bf16(out.reshape(B, 2, P, 2500))


def _wtap_lhsT(Wtap, branch, g):
    """Pack lhsT layout [2, 128, 9*128] for a branch restricted to quad g."""
    rows = slice(g * P, (g + 1) * P)
    out = np.empty((2, P, 9 * P), dtype=np.float32)
    for kc in range(2):
        for tap in range(9):
            blk = Wtap[branch, tap][rows, kc * P:(kc + 1) * P]
            out[kc, :, tap * P:(tap + 1) * P] = blk.T
    return _bf16(out)


def kernel(x1, x2, dw_w, bn_gamma, bn_beta, bn_mean, bn_var, pw_w, pw_b, lin_w,
           h1=HW, w1=HW, h2=HW, w2=HW):
    global _PROGRAM
    from concourse.bass_utils import run_bass_kernel_spmd

    x1 = np.asarray(x1, dtype=np.float32)
    x2 = np.asarray(x2, dtype=np.float32)

    Wtap, c = _fold_weights(np.asarray(dw_w), np.asarray(bn_gamma),
                            np.asarray(bn_beta), np.asarray(bn_mean),
                            np.asarray(bn_var), np.asarray(pw_w),
                            np.asarray(pw_b), np.asarray(lin_w))
    pad1 = _pad_images(x1)
    pad2 = _pad_images(x2)

    if _PROGRAM is None:
        _PROGRAM = _build_program()
    nc = _PROGRAM

    # core layout: core = m*4 + b*2 + g
    in_maps = []
    for m in range(2):
        qbr, kbr, vbr = (0, 4, 5) if m == 0 else (3, 1, 2)
        pa, pb_ = (pad1, pad2) if m == 0 else (pad2, pad1)
        for b in range(2):
            for g in range(2):
                bias = np.stack([c[qbr, g * P:(g + 1) * P],
                                 c[kbr, g * P:(g + 1) * P],
                                 c[vbr, g * P:(g + 1) * P]])[:, :, None]
                in_maps.append({
                    "pad_a": np.ascontiguousarray(pa[b]),
                    "pad_b": np.ascontiguousarray(pb_[b]),
                    "wq": _wtap_lhsT(Wtap, qbr, g),
                    "wk": _wtap_lhsT(Wtap, kbr, g),
                    "wv": _wtap_lhsT(Wtap, vbr, g),
                    "bias": np.ascontiguousarray(bias),
                })

    global _last_in_maps
    _last_in_maps = in_maps
    res = run_bass_kernel_spmd(nc, in_maps, list(range(N_CORES)))

    o = np.empty((2, 2, HEADS, T, DH), dtype=np.float32)
    for m in range(2):
        for b in range(2):
            for g in range(2):
                core = m * 4 + b * 2 + g
                att = res.results[core]["out"]
                qf = res.results[core]["qout"]
                a = att.reshape(P, N_LT, 2, 4, DH).transpose(1, 2, 0, 3, 4)
                a = np.ascontiguousarray(a.reshape(T, 4, DH))
                avl = res.results[core]["avlast"].reshape(P, 2, 4, 33)
                blk = avl[:, :, :, :DH] / avl[:, :, :, DH:DH + 1]
                a[(N_LT - 1) * NL:] = blk.transpose(1, 0, 2, 3).reshape(
                    NL, 4, DH)
                qq = qf.reshape(4, DH, T).transpose(2, 0, 1)
                o[m, b, 4 * g:4 * g + 4] = (a + qq).transpose(1, 0, 2)
    o1 = o[0].reshape(B, T, HEADS * DH)
    o2 = o[1].reshape(B, T, HEADS * DH)
    return o1, o2
